# revision 3
# baseline (speedup 1.0000x reference)
"""Causal self-attention on 8 TRN2 NeuronCores.

Sharding: core c handles batch b=c//2, head-group g=c%2 (heads g*8..g*8+7).
Each core computes the qkv projection for its 8 heads, causal attention, and
a partial out-projection (its heads' columns of w_out). Host sums the two
partial outputs per batch. All layout transposes are done host-side.

On-chip (per core), P=128 partitions, bf16 matmul operands, f32 PSUM:
  xT    [1024(c), 2048(t)]   x[b] transposed
  wqkvT [1024(c), 1536(f)]   f = [qT 512 | kT 512 | vT 512] for this group
  woutT [512(dv), 1024(o)]   w_out columns for this group, transposed
  scoresT[j, i] = sum_d kT[d,j] qT[d,i]  (softmax runs over partition dim j)
  exp via ACT with additive -1e30 causal mask on the diagonal tiles; the
  softmax denominator is produced by the same PV matmul via a 64-wide ones
  block appended to v (psum rows 64:128 all hold sum_j p[j,i]).
"""

import math
import numpy as np
import ml_dtypes

B, T, D, H, HD = 4, 2048, 1024, 16, 64
P = 128
HPG = 8          # heads per group
FG = HPG * HD    # 512 features per group
NCC = D // P     # 8 contraction chunks
NTB = 4          # t-blocks of 512
NTT = 16         # t-tiles of 128
NIB = 4          # i-blocks of 512
SCALE = 1.0 / math.sqrt(HD)
NEG = -1.0e30

_CACHE = {}


def _build_nc():
    from concourse import bacc
    import concourse.mybir as mybir
    import concourse.tile as tile
    from contextlib import ExitStack

    BF = mybir.dt.bfloat16
    F32 = mybir.dt.float32

    nc = bacc.Bacc("TRN2", target_bir_lowering=False, debug=False, num_devices=8)
    xT = nc.dram_tensor("xT", [D, T], BF, kind="ExternalInput").ap()
    wqkvT = nc.dram_tensor("wqkvT", [D, 3 * FG], BF, kind="ExternalInput").ap()
    woutT = nc.dram_tensor("woutT", [FG, D], BF, kind="ExternalInput").ap()
    maskd = nc.dram_tensor("maskd", [4, P, 512], F32, kind="ExternalInput").ap()
    out = nc.dram_tensor("out", [T, D], F32, kind="ExternalOutput").ap()

    with tile.TileContext(nc) as tc, ExitStack() as ctx:
        singles = ctx.enter_context(tc.tile_pool(name="singles", bufs=1))
        xtp = ctx.enter_context(tc.tile_pool(name="xt", bufs=2))
        ptp = ctx.enter_context(tc.tile_pool(name="pt", bufs=4))
        rcp = ctx.enter_context(tc.tile_pool(name="rc", bufs=4))
        yp = ctx.enter_context(tc.tile_pool(name="y", bufs=3))
        ps_mm = ctx.enter_context(tc.tile_pool(name="ps_mm", bufs=2, space="PSUM"))
        ps_qk = ctx.enter_context(tc.tile_pool(name="ps_qk", bufs=3, space="PSUM"))
        ps_pv = ctx.enter_context(tc.tile_pool(name="ps_pv", bufs=2, space="PSUM"))

        wq_sb = singles.tile([P, NCC, 3 * FG], BF)
        nc.sync.dma_start(out=wq_sb, in_=wqkvT.rearrange("(cc p) f -> p cc f", p=P))
        wo_sb = singles.tile([P, 4, D], BF)
        nc.sync.dma_start(out=wo_sb, in_=woutT.rearrange("(dc p) o -> p dc o", p=P))
        mask_sb = singles.tile([P, 4, 512], F32)
        for r in range(4):
            nc.sync.dma_start(out=mask_sb[:, r, :], in_=maskd[r])

        qk_sb = singles.tile([P, 8, T], BF)              # f-tiles 0..3 = q, 4..7 = k
        vp_sb = singles.tile([P, NTT, HPG, 2 * HD], BF)  # [v_h | ones(64)]
        oT_sb = singles.tile([P, 4, T], BF)              # attn out, [dv, t]
        nc.vector.memset(vp_sb[:, :, :, HD:2 * HD], 1.0)

        # ---- QKV projection ----
        for tb in range(NTB):
            xt = xtp.tile([P, NCC, 512], BF)
            nc.sync.dma_start(
                out=xt,
                in_=xT[:, tb * 512:(tb + 1) * 512].rearrange("(cc p) t -> p cc t", p=P),
            )
            for ft in range(8):  # q then k feature tiles, output [f=128, t=512]
                ps = ps_mm.tile([P, 512], mybir.dt.float32)
                for cc in range(NCC):
                    nc.tensor.matmul(
                        ps,
                        lhsT=wq_sb[:, cc, ft * P:(ft + 1) * P],
                        rhs=xt[:, cc, :],
                        start=(cc == 0),
                        stop=(cc == NCC - 1),
                    )
                nc.vector.tensor_copy(
                    out=qk_sb[:, ft, tb * 512:(tb + 1) * 512], in_=ps
                )
            for tl in range(4):  # v in [t, dv] orientation, output [t=128, dv=512]
                tt = tb * 4 + tl
                ps = ps_mm.tile([P, FG], mybir.dt.float32)
                for cc in range(NCC):
                    nc.tensor.matmul(
                        ps,
                        lhsT=xt[:, cc, tl * P:(tl + 1) * P],
                        rhs=wq_sb[:, cc, 2 * FG:3 * FG],
                        start=(cc == 0),
                        stop=(cc == NCC - 1),
                    )
                nc.vector.tensor_copy(
                    out=vp_sb[:, tt, :, 0:HD],
                    in_=ps.rearrange("p (h d) -> p h d", h=HPG),
                )

        # ---- attention + out-projection, per i-block ----
        for ib in range(NIB):
            isl = slice(ib * 512, (ib + 1) * 512)
            for h in range(HPG):
                po = (h % 2) * 64
                fq = h // 2
                fk = 4 + h // 2
                pv = ps_pv.tile([P, 512], mybir.dt.float32)
                njt = 4 * ib + 4
                for jt in range(njt):
                    qk = ps_qk.tile([P, 512], mybir.dt.float32)
                    nc.tensor.matmul(
                        qk,
                        lhsT=qk_sb[po:po + 64, fk, jt * P:(jt + 1) * P],
                        rhs=qk_sb[po:po + 64, fq, isl],
                        start=True,
                        stop=True,
                    )
                    r = jt - 4 * ib
                    if r >= 0:
                        nc.vector.tensor_add(qk, qk, mask_sb[:, r, :])
                    pt = ptp.tile([P, 512], BF)
                    nc.scalar.activation(
                        out=pt, in_=qk,
                        func=mybir.ActivationFunctionType.Exp, scale=SCALE,
                    )
                    nc.tensor.matmul(
                        pv,
                        lhsT=vp_sb[:, jt, h, :],
                        rhs=pt,
                        start=(jt == 0),
                        stop=(jt == njt - 1),
                    )
                rc = rcp.tile([64, 512], mybir.dt.float32)
                nc.vector.reciprocal(rc, pv[64:P, :])
                nc.vector.tensor_mul(
                    oT_sb[po:po + 64, h // 2, isl], pv[0:64, :], rc
                )
            # out-projection for this i-block's four t-tiles
            for tt in range(4 * ib, 4 * ib + 4):
                for ob in range(2):
                    ps = ps_mm.tile([P, 512], mybir.dt.float32)
                    for dc in range(4):
                        nc.tensor.matmul(
                            ps,
                            lhsT=oT_sb[:, dc, tt * P:(tt + 1) * P],
                            rhs=wo_sb[:, dc, ob * 512:(ob + 1) * 512],
                            start=(dc == 0),
                            stop=(dc == 3),
                        )
                    yt = yp.tile([P, 512], mybir.dt.float32)
                    nc.vector.tensor_copy(yt, ps)
                    nc.sync.dma_start(
                        out=out[tt * P:(tt + 1) * P, ob * 512:(ob + 1) * 512],
                        in_=yt,
                    )
    nc.compile()
    return nc


def _make_in_maps(x, w_qkv, w_out):
    bf = ml_dtypes.bfloat16
    mask = np.where(
        np.arange(512)[None, None, :]
        >= (np.arange(P)[None, :, None] + P * np.arange(4)[:, None, None]),
        np.float32(0.0), np.float32(NEG),
    ).astype(np.float32)  # [4, 128, 512]
    in_maps = []
    for c in range(8):
        b, g = c // 2, c % 2
        wq = w_qkv[g * FG:(g + 1) * FG]
        wk = w_qkv[D + g * FG:D + (g + 1) * FG]
        wv = w_qkv[2 * D + g * FG:2 * D + (g + 1) * FG]
        in_maps.append({
            "xT": np.ascontiguousarray(x[b].T).astype(bf),
            "wqkvT": np.ascontiguousarray(
                np.concatenate([wq.T, wk.T, wv.T], axis=1)).astype(bf),
            "woutT": np.ascontiguousarray(w_out[:, g * FG:(g + 1) * FG].T).astype(bf),
            "maskd": mask,
        })
    return in_maps


def _ensure_ntff_hook():
    """The agent image's antenv package lacks axon_hooks; shim it so
    run_bass_kernel_spmd(trace=True) can capture NTFF profiles."""
    import sys, types
    try:
        import antenv.axon_hooks  # noqa: F401
        return
    except ImportError:
        pass
    import antenv
    mod = types.ModuleType("antenv.axon_hooks")
    mod._hook = None
    def set_axon_ntff_profile_hook(h):
        mod._hook = h
    def get_axon_ntff_profile_hook():
        return mod._hook
    mod.set_axon_ntff_profile_hook = set_axon_ntff_profile_hook
    mod.get_axon_ntff_profile_hook = get_axon_ntff_profile_hook
    sys.modules["antenv.axon_hooks"] = mod
    antenv.axon_hooks = mod
    try:
        from trn_agent_boot.trn_boot import _ntff_profile_via_ctypes
        set_axon_ntff_profile_hook(
            _ntff_profile_via_ctypes("/opt/axon/libaxon_pjrt.so"))
    except Exception as e:  # degrade to no tracing
        print(f"ntff hook install failed: {e}")


def run(x, w_qkv, w_out, trace=False, trace_kwargs=None):
    if trace:
        _ensure_ntff_hook()
    from concourse.bass_utils import run_bass_kernel_spmd

    if "nc" not in _CACHE:
        _CACHE["nc"] = _build_nc()
    nc = _CACHE["nc"]
    in_maps = _make_in_maps(np.asarray(x), np.asarray(w_qkv), np.asarray(w_out))
    kw = dict(trace_kwargs or {})
    res = run_bass_kernel_spmd(nc, in_maps, core_ids=list(range(8)), trace=trace, **kw)
    outs = [r["out"] for r in res.results]
    full = np.empty((B, T, D), dtype=np.float32)
    for b in range(B):
        full[b] = outs[2 * b].astype(np.float32) + outs[2 * b + 1].astype(np.float32)
    return full, res


def kernel(x, w_qkv, w_out):
    full, _ = run(x, w_qkv, w_out, trace=False)
    return full


# revision 10
# speedup vs baseline: 1.0958x; 1.0958x over previous
"""Causal self-attention on 8 TRN2 NeuronCores.

Sharding: core c handles batch b=c//2, head-group g=c%2 (heads g*8..g*8+7).
Each core computes the qkv projection for its 8 heads, causal attention, and
a partial out-projection (its heads' columns of w_out). Host sums the two
partial outputs per batch. All layout transposes are done host-side.

On-chip (per core), P=128 partitions, bf16 matmul operands, f32 PSUM:
  xT    [1024(c), 2048(t)]   x[b] transposed
  wqkvT [1024(c), 1536(f)]   f = [qT 512 | kT 512 | vT 512] for this group
  woutT [512(dv), 1024(o)]   w_out columns for this group, transposed
  scoresT[j, i] = sum_d kT[d,j] qT[d,i]  (softmax runs over partition dim j)
  exp via ACT with additive -1e30 causal mask on the diagonal tiles; the
  softmax denominator is produced by the same PV matmul via a 64-wide ones
  block appended to v (psum rows 64:128 all hold sum_j p[j,i]).
"""

import math
import numpy as np
import ml_dtypes

B, T, D, H, HD = 4, 2048, 1024, 16, 64
P = 128
HPG = 8          # heads per group
FG = HPG * HD    # 512 features per group
NCC = D // P     # 8 contraction chunks
NTB = 4          # t-blocks of 512
NTT = 16         # t-tiles of 128
NIB = 4          # i-blocks of 512
SCALE = 1.0 / math.sqrt(HD)
NEG = -1.0e30

_CACHE = {}


def _build_nc():
    from concourse import bacc
    import concourse.mybir as mybir
    import concourse.tile as tile
    from contextlib import ExitStack

    BF = mybir.dt.bfloat16
    F32 = mybir.dt.float32

    nc = bacc.Bacc("TRN2", target_bir_lowering=False, debug=False, num_devices=8)
    xT = nc.dram_tensor("xT", [D, T], BF, kind="ExternalInput").ap()
    wqkvT = nc.dram_tensor("wqkvT", [D, 3 * FG], BF, kind="ExternalInput").ap()
    woutT = nc.dram_tensor("woutT", [FG, D], BF, kind="ExternalInput").ap()
    maskd = nc.dram_tensor("maskd", [P, P], F32, kind="ExternalInput").ap()
    out = nc.dram_tensor("out", [T, D], F32, kind="ExternalOutput").ap()

    with tile.TileContext(nc) as tc, ExitStack() as ctx:
        singles = ctx.enter_context(tc.tile_pool(name="singles", bufs=1))
        xtp = ctx.enter_context(tc.tile_pool(name="xt", bufs=2))
        ptp = ctx.enter_context(tc.tile_pool(name="pt", bufs=6))
        rcp = ctx.enter_context(tc.tile_pool(name="rc", bufs=4))
        yp = ctx.enter_context(tc.tile_pool(name="y", bufs=3))
        ps_mm = ctx.enter_context(tc.tile_pool(name="ps_mm", bufs=2, space="PSUM"))
        ps_qk = ctx.enter_context(tc.tile_pool(name="ps_qk", bufs=4, space="PSUM"))
        ps_pv = ctx.enter_context(tc.tile_pool(name="ps_pv", bufs=2, space="PSUM"))

        wq_sb = singles.tile([P, NCC, 3 * FG], BF)
        nc.sync.dma_start(out=wq_sb, in_=wqkvT.rearrange("(cc p) f -> p cc f", p=P))
        wo_sb = singles.tile([P, 4, D], BF)
        nc.sync.dma_start(out=wo_sb, in_=woutT.rearrange("(dc p) o -> p dc o", p=P))
        mask_sb = singles.tile([P, P], F32)
        nc.sync.dma_start(out=mask_sb, in_=maskd)

        qk_sb = singles.tile([P, 8, T], BF)              # f-tiles 0..3 = q, 4..7 = k
        vp_sb = singles.tile([P, NTT, HPG, 2 * HD], BF)  # [v_h | ones(64)]
        oT_sb = singles.tile([P, 4, T], BF)              # attn out, [dv, t]
        nc.vector.memset(vp_sb[:, :, :, HD:2 * HD], 1.0)

        # ---- QKV projection ----
        for tb in range(NTB):
            xt = xtp.tile([P, NCC, 512], BF)
            nc.sync.dma_start(
                out=xt,
                in_=xT[:, tb * 512:(tb + 1) * 512].rearrange("(cc p) t -> p cc t", p=P),
            )
            for ft in range(8):  # q then k feature tiles, output [f=128, t=512]
                ps = ps_mm.tile([P, 512], mybir.dt.float32)
                for cc in range(NCC):
                    nc.tensor.matmul(
                        ps,
                        lhsT=wq_sb[:, cc, ft * P:(ft + 1) * P],
                        rhs=xt[:, cc, :],
                        start=(cc == 0),
                        stop=(cc == NCC - 1),
                    )
                nc.scalar.copy(
                    out=qk_sb[:, ft, tb * 512:(tb + 1) * 512], in_=ps
                )
            for tl in range(4):  # v in [t, dv] orientation, output [t=128, dv=512]
                tt = tb * 4 + tl
                ps = ps_mm.tile([P, FG], mybir.dt.float32)
                for cc in range(NCC):
                    nc.tensor.matmul(
                        ps,
                        lhsT=xt[:, cc, tl * P:(tl + 1) * P],
                        rhs=wq_sb[:, cc, 2 * FG:3 * FG],
                        start=(cc == 0),
                        stop=(cc == NCC - 1),
                    )
                nc.scalar.copy(
                    out=vp_sb[:, tt, :, 0:HD],
                    in_=ps.rearrange("p (h d) -> p h d", h=HPG),
                )

        # ---- attention + out-projection, per i-block ----
        for ib in range(NIB):
            isl = slice(ib * 512, (ib + 1) * 512)
            for h in range(HPG):
                po = (h % 2) * 64
                fq = h // 2
                fk = 4 + h // 2
                pv = ps_pv.tile([P, 512], mybir.dt.float32)
                njt = 4 * ib + 4
                for jt in range(njt):
                    r = jt - 4 * ib
                    c0 = P * r if r > 0 else 0  # valid column start in i-block
                    qk = ps_qk.tile([P, 512], mybir.dt.float32)
                    nc.tensor.matmul(
                        qk[:, c0:512],
                        lhsT=qk_sb[po:po + 64, fk, jt * P:(jt + 1) * P],
                        rhs=qk_sb[po:po + 64, fq, ib * 512 + c0:(ib + 1) * 512],
                        start=True,
                        stop=True,
                    )
                    if r >= 0:  # mask the diagonal 128x128 sub-block
                        nc.vector.tensor_add(
                            qk[:, c0:c0 + P], qk[:, c0:c0 + P], mask_sb
                        )
                    pt = ptp.tile([P, 512], BF)
                    nc.scalar.activation(
                        out=pt[:, c0:512], in_=qk[:, c0:512],
                        func=mybir.ActivationFunctionType.Exp, scale=SCALE,
                    )
                    nc.tensor.matmul(
                        pv[:, c0:512],
                        lhsT=vp_sb[:, jt, h, :],
                        rhs=pt[:, c0:512],
                        start=(jt == 0),
                        stop=(jt == njt - 1),
                    )
                rc = rcp.tile([64, 512], mybir.dt.float32)
                nc.vector.reciprocal(rc, pv[64:P, :])
                nc.vector.tensor_mul(
                    oT_sb[po:po + 64, h // 2, isl], pv[0:64, :], rc
                )
            # out-projection for this i-block's four t-tiles
            for tt in range(4 * ib, 4 * ib + 4):
                for ob in range(2):
                    ps = ps_mm.tile([P, 512], mybir.dt.float32)
                    for dc in range(4):
                        nc.tensor.matmul(
                            ps,
                            lhsT=oT_sb[:, dc, tt * P:(tt + 1) * P],
                            rhs=wo_sb[:, dc, ob * 512:(ob + 1) * 512],
                            start=(dc == 0),
                            stop=(dc == 3),
                        )
                    yt = yp.tile([P, 512], mybir.dt.float32)
                    nc.vector.tensor_copy(yt, ps)
                    nc.sync.dma_start(
                        out=out[tt * P:(tt + 1) * P, ob * 512:(ob + 1) * 512],
                        in_=yt,
                    )
    nc.compile()
    return nc


def _make_in_maps(x, w_qkv, w_out):
    bf = ml_dtypes.bfloat16
    # triangular mask for the diagonal 128x128 block: keep i_local >= j_local
    mask = np.where(
        np.arange(P)[None, :] >= np.arange(P)[:, None],
        np.float32(0.0), np.float32(NEG),
    ).astype(np.float32)  # [128, 128]
    in_maps = []
    for c in range(8):
        b, g = c // 2, c % 2
        wq = w_qkv[g * FG:(g + 1) * FG]
        wk = w_qkv[D + g * FG:D + (g + 1) * FG]
        wv = w_qkv[2 * D + g * FG:2 * D + (g + 1) * FG]
        in_maps.append({
            "xT": np.ascontiguousarray(x[b].T).astype(bf),
            "wqkvT": np.ascontiguousarray(
                np.concatenate([wq.T, wk.T, wv.T], axis=1)).astype(bf),
            "woutT": np.ascontiguousarray(w_out[:, g * FG:(g + 1) * FG].T).astype(bf),
            "maskd": mask,
        })
    return in_maps


def _ensure_ntff_hook():
    """The agent image's antenv package lacks axon_hooks; shim it so
    run_bass_kernel_spmd(trace=True) can capture NTFF profiles."""
    import sys, types
    try:
        import antenv.axon_hooks  # noqa: F401
        return
    except ImportError:
        pass
    import antenv
    mod = types.ModuleType("antenv.axon_hooks")
    mod._hook = None
    def set_axon_ntff_profile_hook(h):
        mod._hook = h
    def get_axon_ntff_profile_hook():
        return mod._hook
    mod.set_axon_ntff_profile_hook = set_axon_ntff_profile_hook
    mod.get_axon_ntff_profile_hook = get_axon_ntff_profile_hook
    sys.modules["antenv.axon_hooks"] = mod
    antenv.axon_hooks = mod
    try:
        from trn_agent_boot.trn_boot import _ntff_profile_via_ctypes
        set_axon_ntff_profile_hook(
            _ntff_profile_via_ctypes("/opt/axon/libaxon_pjrt.so"))
    except Exception as e:  # degrade to no tracing
        print(f"ntff hook install failed: {e}")


def run(x, w_qkv, w_out, trace=False, trace_kwargs=None):
    if trace:
        _ensure_ntff_hook()
    from concourse.bass_utils import run_bass_kernel_spmd

    if "nc" not in _CACHE:
        _CACHE["nc"] = _build_nc()
    nc = _CACHE["nc"]
    in_maps = _make_in_maps(np.asarray(x), np.asarray(w_qkv), np.asarray(w_out))
    kw = dict(trace_kwargs or {})
    res = run_bass_kernel_spmd(nc, in_maps, core_ids=list(range(8)), trace=trace, **kw)
    outs = [r["out"] for r in res.results]
    full = np.empty((B, T, D), dtype=np.float32)
    for b in range(B):
        full[b] = outs[2 * b].astype(np.float32) + outs[2 * b + 1].astype(np.float32)
    return full, res


def kernel(x, w_qkv, w_out):
    full, _ = run(x, w_qkv, w_out, trace=False)
    return full
